# revision 9
# baseline (speedup 1.0000x reference)
"""Trainium2 Bass kernel for a dense transformer block.

Reference computation (per batch element):
    x = x + attn(LN1(x));  out = x + MLP(LN2(x))
with B=8, T=1024, C=1024, 16 heads, causal attention, GELU(tanh) MLP.

Sharding: pure data-parallel over batch — B=8 exactly matches the 8
NeuronCores, so each core runs the full block on its own [T, C] slice with
replicated weights.  No collectives needed.

Device strategy (per core):
  - LN1/LN2 affine params are folded into the following matmul weights on
    the host (exact linear algebra); biases are emitted on device only when
    nonzero (the kernel re-traces per distinct bias-nonzero pattern).
  - All matmuls run in bf16 with fp32 PSUM accumulation.
  - Attention is computed via a transposed-score layout: scoresT[k, q] tiles
    with k on partitions, so softmax needs no cross-partition reduction.
    The softmax denominator comes free from an appended ones-column on V
    (AV matmul rhs = [V | 1]); division is a per-partition scalar multiply.
  - Causal masking: per k-tile only columns q >= 128*kt are computed; the
    single [128,128] diagonal triangle is masked by multiplying the exp()
    output with a precomputed upper-triangular 0/1 tile.
  - exp uses a constant shift instead of a row max (scores are bounded for
    any realistic input scale; the shift cancels exactly in the ratio).
  - SBUF is tight (~207 KB/partition): large tensors share slots via tag
    chains whose lifetimes are disjoint (see tags slotA/slotB/hTr/shared16).
"""

from contextlib import ExitStack

import numpy as np
import ml_dtypes

import concourse.bass as bass
import concourse.mybir as mybir
import concourse.tile as tile
from concourse import bacc
from concourse.bass_utils import run_bass_kernel_spmd
from concourse.masks import make_identity, make_upper_triangular

F32 = mybir.dt.float32
BF16 = mybir.dt.bfloat16
AF = mybir.ActivationFunctionType
ALU = mybir.AluOpType

N_CORES = 8
T = 1024
C = 1024
NH = 16
HS = 64
H4 = 4 * C
EPS = 1e-5
EXP_SHIFT = 20.0  # exp(s/8 - 20): cancels in softmax ratio, guards fp32 overflow

TT = T // 128   # 8 token tiles
CT = C // 128   # 8 feature tiles
MT = 2 * C // 128  # 16 q+k feature tiles
HM = H4 // 128  # 32 hidden tiles
PJ = 256        # fc2 output column chunk


def _emit(ctx, tc, xd, wqkvd, wod, wfcd, wprojd, outd, biases):
    """Emit the full block for one core. biases: dict name -> dram AP."""
    nc = tc.nc

    singles = ctx.enter_context(tc.tile_pool(name="singles", bufs=1))
    big = ctx.enter_context(tc.tile_pool(name="big", bufs=1))
    stage = ctx.enter_context(tc.tile_pool(name="stage", bufs=2))
    stats = ctx.enter_context(tc.tile_pool(name="stats", bufs=6))
    weip = ctx.enter_context(tc.tile_pool(name="weip", bufs=1))
    outst = ctx.enter_context(tc.tile_pool(name="outst", bufs=2))
    ps512 = ctx.enter_context(tc.tile_pool(name="ps512", bufs=3, space="PSUM"))
    psav = ctx.enter_context(tc.tile_pool(name="psav", bufs=2, space="PSUM"))
    pstr = ctx.enter_context(tc.tile_pool(name="pstr", bufs=2, space="PSUM"))

    eps_t = singles.tile([128, 1], F32, tag="eps")
    nc.vector.memset(eps_t, EPS)
    id32 = singles.tile([128, 128], F32, tag="id32")
    make_identity(nc, id32)
    id16 = singles.tile([128, 128], BF16, tag="id16")
    make_identity(nc, id16)
    tri = singles.tile([128, 128], BF16, tag="tri")
    make_upper_triangular(nc, tri, val=1.0, diag=True)
    shift_t = singles.tile([128, 1], F32, tag="shift")
    nc.vector.memset(shift_t, -EXP_SHIFT)

    def ln_transpose(src_of_tt, dst):
        """LayerNorm (no affine) + PE-transpose to feature-major bf16.

        src_of_tt(tt) -> [128, C] f32 AP; dst: [128, CT, T] bf16."""
        for tt in range(TT):
            xs = src_of_tt(tt)
            st = stats.tile([128, 2, 6], F32, tag="bst")
            for g in range(2):
                nc.vector.bn_stats(out=st[:, g], in_=xs[:, g * 512:(g + 1) * 512])
            mv = stats.tile([128, 2], F32, tag="mv")
            nc.vector.bn_aggr(out=mv, in_=st)
            rstd = stats.tile([128, 1], F32, tag="rstd")
            nc.scalar.activation(rstd, mv[:, 1:2], AF.Sqrt, bias=eps_t)
            nc.vector.reciprocal(rstd, rstd)
            hst = stage.tile([128, C], F32, tag="lnst")
            nc.vector.tensor_scalar(
                out=hst, in0=xs, scalar1=mv[:, 0:1], scalar2=rstd,
                op0=ALU.subtract, op1=ALU.mult)
            for ct in range(CT):
                ps = pstr.tile([128, 128], F32, tag="ptr")
                nc.tensor.transpose(ps, hst[:, ct * 128:(ct + 1) * 128], id32)
                nc.scalar.copy(out=dst[:, ct, tt * 128:(tt + 1) * 128], in_=ps)

    # ---- LN1 -> h1T ----
    h1T = big.tile([128, CT, T], BF16, tag="hTr")

    def x_tile(tt):
        xt = stage.tile([128, C], F32, tag="xt")
        nc.sync.dma_start(out=xt, in_=xd[tt * 128:(tt + 1) * 128, :])
        return xt

    ln_transpose(x_tile, h1T)

    # ---- QKV ----
    qkT = big.tile([128, MT, T], BF16, tag="slotB")
    vext = big.tile([128, TT, NH, HS + 1], BF16, tag="vext")
    nc.vector.memset(vext[:, :, :, HS:HS + 1], 1.0)
    wqkv_r = wqkvd.rearrange("(ct p) n -> p ct n", p=128)

    for m in range(MT):
        wm = big.tile([128, CT, 128], BF16, tag="wsm", bufs=2)
        nc.sync.dma_start(out=wm, in_=wqkv_r[:, :, m * 128:(m + 1) * 128])
        bq_t = None
        if "bqkv" in biases:
            bq_t = stats.tile([128, 1], F32, tag="bq")
            nc.sync.dma_start(
                out=bq_t,
                in_=biases["bqkv"][m * 128:(m + 1) * 128].rearrange("(p o) -> p o", o=1))
        for qc in range(2):
            ps = ps512.tile([128, 512], F32, tag="mm")
            for ct in range(CT):
                nc.tensor.matmul(ps, wm[:, ct], h1T[:, ct, qc * 512:(qc + 1) * 512],
                                 start=(ct == 0), stop=(ct == CT - 1))
            dst = qkT[:, m, qc * 512:(qc + 1) * 512]
            if bq_t is not None:
                nc.vector.tensor_scalar_add(out=dst, in0=ps, scalar1=bq_t)
            else:
                nc.scalar.copy(out=dst, in_=ps)

    for vc in range(2):
        wv = big.tile([128, CT, 512], BF16, tag="shared16", bufs=2)
        nc.sync.dma_start(out=wv, in_=wqkv_r[:, :, 2 * C + vc * 512: 2 * C + (vc + 1) * 512])
        bv_t = None
        if "bqkv" in biases:
            bv_t = stage.tile([128, 512], F32, tag="bv")
            nc.sync.dma_start(
                out=bv_t,
                in_=biases["bqkv"][2 * C + vc * 512: 2 * C + (vc + 1) * 512].to_broadcast((128, 512)))
        for tt in range(TT):
            ps = ps512.tile([128, 512], F32, tag="mm")
            for ct in range(CT):
                nc.tensor.matmul(ps, h1T[:, ct, tt * 128:(tt + 1) * 128], wv[:, ct],
                                 start=(ct == 0), stop=(ct == CT - 1))
            if bv_t is not None:
                nc.vector.tensor_add(out=ps, in0=ps, in1=bv_t)
            dstv = vext[:, tt, vc * 8:(vc + 1) * 8, 0:HS]
            nc.scalar.copy(out=dstv, in_=ps.rearrange("p (h e) -> p h e", e=HS))

    # ---- attention ----
    attT = big.tile([128, CT, T], BF16, tag="shared16", bufs=2)
    for h in range(NH):
        po = 64 * (h % 2)
        kT = qkT[po:po + 64, MT // 2 + h // 2]   # [64, T]
        qT = qkT[po:po + 64, h // 2]             # [64, T]
        wts = []
        for kt in range(TT):
            w_kt = weip.tile([128, T - kt * 128], BF16, tag=f"wei{kt}")
            wts.append(w_kt)
            q0 = kt * 128
            while q0 < T:
                w = min(512, T - q0)
                ps = ps512.tile([128, 512], F32, tag="mm")
                nc.tensor.matmul(ps[:, :w], kT[:, kt * 128:(kt + 1) * 128],
                                 qT[:, q0:q0 + w], start=True, stop=True)
                nc.scalar.activation(out=w_kt[:, q0 - kt * 128: q0 - kt * 128 + w],
                                     in_=ps[:, :w], func=AF.Exp,
                                     scale=1.0 / float(np.sqrt(HS)), bias=shift_t)
                q0 += w
            nc.vector.tensor_mul(out=w_kt[:, 0:128], in0=w_kt[:, 0:128], in1=tri)
        for qt in range(TT):
            pav = psav.tile([128, HS + 1], F32, tag="av")
            for kt in range(qt + 1):
                nc.tensor.matmul(pav, wts[kt][:, (qt - kt) * 128:(qt - kt) * 128 + 128],
                                 vext[:, kt, h], start=(kt == 0), stop=(kt == qt))
            inv = stats.tile([128, 1], F32, tag="inv")
            nc.vector.reciprocal(inv, pav[:, HS:HS + 1])
            ast = stage.tile([128, HS], BF16, tag="ast", bufs=4)
            nc.vector.tensor_scalar_mul(out=ast, in0=pav[:, 0:HS], scalar1=inv)
            ptr = pstr.tile([HS, 128], BF16, tag="ptr_bf", bufs=1)
            nc.tensor.transpose(ptr, ast, id16)
            nc.scalar.copy(out=attT[po:po + 64, h // 2, qt * 128:(qt + 1) * 128], in_=ptr)

    # ---- attention out projection + residual ----
    x2 = big.tile([128, TT, C], F32, tag="slotB")
    wo_t = big.tile([128, CT, C], BF16, tag="hTr")
    nc.sync.dma_start(out=wo_t, in_=wod.rearrange("(ft p) n -> p ft n", p=128))
    bo_t = None
    if "bo" in biases:
        bo_t = stage.tile([128, C], F32, tag="bo")
        nc.sync.dma_start(out=bo_t, in_=biases["bo"].to_broadcast((128, C)))
    for tt in range(TT):
        xr = stage.tile([128, C], F32, tag="xt")
        nc.sync.dma_start(out=xr, in_=xd[tt * 128:(tt + 1) * 128, :])
        for nk in range(2):
            ps = ps512.tile([128, 512], F32, tag="mm")
            for ft in range(CT):
                nc.tensor.matmul(ps, attT[:, ft, tt * 128:(tt + 1) * 128],
                                 wo_t[:, ft, nk * 512:(nk + 1) * 512],
                                 start=(ft == 0), stop=(ft == CT - 1))
            dst = x2[:, tt, nk * 512:(nk + 1) * 512]
            nc.vector.tensor_add(out=dst, in0=ps, in1=xr[:, nk * 512:(nk + 1) * 512])
            if bo_t is not None:
                nc.vector.tensor_add(out=dst, in0=dst, in1=bo_t[:, nk * 512:(nk + 1) * 512])

    # ---- LN2 -> h2T ----
    h2T = big.tile([128, CT, T], BF16, tag="hTr")
    ln_transpose(lambda tt: x2[:, tt], h2T)

    # ---- fc1 + gelu ----
    hbig = big.tile([128, HM, T], BF16, tag="slotA")
    wfc_r = wfcd.rearrange("(ct p) n -> p ct n", p=128)
    for m in range(HM):
        wm = big.tile([128, CT, 128], BF16, tag="wsm", bufs=2)
        nc.sync.dma_start(out=wm, in_=wfc_r[:, :, m * 128:(m + 1) * 128])
        bf_t = 0.0
        if "bfc" in biases:
            bf_t = stats.tile([128, 1], F32, tag="bf")
            nc.sync.dma_start(
                out=bf_t,
                in_=biases["bfc"][m * 128:(m + 1) * 128].rearrange("(p o) -> p o", o=1))
        for qc in range(2):
            ps = ps512.tile([128, 512], F32, tag="mm")
            for ct in range(CT):
                nc.tensor.matmul(ps, wm[:, ct], h2T[:, ct, qc * 512:(qc + 1) * 512],
                                 start=(ct == 0), stop=(ct == CT - 1))
            nc.scalar.activation(out=hbig[:, m, qc * 512:(qc + 1) * 512], in_=ps,
                                 func=AF.Gelu_apprx_tanh, bias=bf_t)

    # ---- fc2 + residual -> out ----
    wproj_r = wprojd.rearrange("(ht p) n -> p ht n", p=128)
    bp_t = None
    if "bproj" in biases:
        bp_t = stage.tile([128, C], F32, tag="bp")
        nc.sync.dma_start(out=bp_t, in_=biases["bproj"].to_broadcast((128, C)))
    for nk in range(C // PJ):
        wp = big.tile([128, HM, PJ], BF16, tag="shared16", bufs=2)
        nc.sync.dma_start(out=wp, in_=wproj_r[:, :, nk * PJ:(nk + 1) * PJ])
        for tt in range(TT):
            ps = ps512.tile([128, PJ], F32, tag="mm")
            for ht in range(HM):
                nc.tensor.matmul(ps, hbig[:, ht, tt * 128:(tt + 1) * 128], wp[:, ht],
                                 start=(ht == 0), stop=(ht == HM - 1))
            ost = outst.tile([128, PJ], F32, tag="ost")
            nc.vector.tensor_add(out=ost, in0=ps, in1=x2[:, tt, nk * PJ:(nk + 1) * PJ])
            if bp_t is not None:
                nc.vector.tensor_add(out=ost, in0=ost, in1=bp_t[:, nk * PJ:(nk + 1) * PJ])
            nc.sync.dma_start(out=outd[tt * 128:(tt + 1) * 128, nk * PJ:(nk + 1) * PJ],
                              in_=ost)


_CACHE = {}


def _build(bias_flags):
    key = bias_flags
    if key in _CACHE:
        return _CACHE[key]
    nc = bacc.Bacc("TRN2", target_bir_lowering=False, debug=False,
                   num_devices=N_CORES)
    xd = nc.dram_tensor("x", [T, C], F32, kind="ExternalInput").ap()
    wqkvd = nc.dram_tensor("wqkv", [C, 3 * C], BF16, kind="ExternalInput").ap()
    wod = nc.dram_tensor("wo", [C, C], BF16, kind="ExternalInput").ap()
    wfcd = nc.dram_tensor("wfc", [C, H4], BF16, kind="ExternalInput").ap()
    wprojd = nc.dram_tensor("wproj", [H4, C], BF16, kind="ExternalInput").ap()
    outd = nc.dram_tensor("out", [T, C], F32, kind="ExternalOutput").ap()
    biases = {}
    has_bqkv, has_bo, has_bfc, has_bproj = bias_flags
    if has_bqkv:
        biases["bqkv"] = nc.dram_tensor("bqkv", [3 * C], F32, kind="ExternalInput").ap()
    if has_bo:
        biases["bo"] = nc.dram_tensor("bo", [C], F32, kind="ExternalInput").ap()
    if has_bfc:
        biases["bfc"] = nc.dram_tensor("bfc", [H4], F32, kind="ExternalInput").ap()
    if has_bproj:
        biases["bproj"] = nc.dram_tensor("bproj", [C], F32, kind="ExternalInput").ap()
    with tile.TileContext(nc) as tc:
        with ExitStack() as ctx:
            _emit(ctx, tc, xd, wqkvd, wod, wfcd, wprojd, outd, biases)
    nc.compile()
    _CACHE[key] = nc
    return nc


def kernel(x, ln1_w, ln1_b, w_qkv, b_qkv, w_o, b_o, ln2_w, ln2_b, w_fc, b_fc,
           w_proj, b_proj):
    x = np.asarray(x, np.float32)
    ln1_w = np.asarray(ln1_w, np.float32)
    ln1_b = np.asarray(ln1_b, np.float32)
    w_qkv = np.asarray(w_qkv, np.float32)
    b_qkv = np.asarray(b_qkv, np.float32)
    w_o = np.asarray(w_o, np.float32)
    b_o = np.asarray(b_o, np.float32)
    ln2_w = np.asarray(ln2_w, np.float32)
    ln2_b = np.asarray(ln2_b, np.float32)
    w_fc = np.asarray(w_fc, np.float32)
    b_fc = np.asarray(b_fc, np.float32)
    w_proj = np.asarray(w_proj, np.float32)
    b_proj = np.asarray(b_proj, np.float32)

    # Fold LN affine params into the adjacent matmuls (exact).
    wqkv_eff = w_qkv * ln1_w[:, None]
    bqkv_eff = ln1_b @ w_qkv + b_qkv
    wfc_eff = w_fc * ln2_w[:, None]
    bfc_eff = ln2_b @ w_fc + b_fc

    bf = ml_dtypes.bfloat16
    wqkv_bf = wqkv_eff.astype(bf)
    wo_bf = w_o.astype(bf)
    wfc_bf = wfc_eff.astype(bf)
    wproj_bf = w_proj.astype(bf)

    flags = (bool(np.any(bqkv_eff)), bool(np.any(b_o)),
             bool(np.any(bfc_eff)), bool(np.any(b_proj)))
    nc = _build(flags)

    in_maps = []
    for b in range(N_CORES):
        m = {"x": x[b], "wqkv": wqkv_bf, "wo": wo_bf, "wfc": wfc_bf,
             "wproj": wproj_bf}
        if flags[0]:
            m["bqkv"] = bqkv_eff
        if flags[1]:
            m["bo"] = b_o
        if flags[2]:
            m["bfc"] = bfc_eff
        if flags[3]:
            m["bproj"] = b_proj
        in_maps.append(m)

    res = run_bass_kernel_spmd(nc, in_maps, list(range(N_CORES)))
    return np.stack([res.results[b]["out"] for b in range(N_CORES)]).astype(np.float32)
